# revision 8
# baseline (speedup 1.0000x reference)
"""Trainium2 Bass kernel for nn_CausalGDM (dense transformer with
vocab-projection softmax attention), 8-way vocab-sharded across 8 NeuronCores.

Sharding: vocab V=32000 padded to 32768, split 8 ways (4096 rows/core).
Each core processes BOTH batches (batch-pipelined to hide all-reduces).
MLP is sharded 8 ways over d_ff. num/den of the gd_step and the MLP partial
sums are combined with AllReduce.

All matmuls run as float32r (TF32-like, full PE rate at N>=256).

SBUF plan (KB/partition, budget ~208):
  res: wteT 64, w1T 4, w2T 4, weff 8, e 16, fT 16, krn2T 8, misc ~1.3 = ~121
  wk tags: arbuf 8, wpec 4, ln_t1 4, exw 2, E 4, Es 4, rows 8, delta_sb 8,
           deltaT/h 8, gT 4, V/xn 8, wt 6, small ~1 = ~70
PSUM plan (banks, budget 8): acc4 (num/delta/dW/mlp/scores) 4,
  zp (Z/g/rows/logits) 2, tr (transp/den/broadcasts) 2.
"""

import sys

sys.path.insert(0, "/opt/trn_rl_repo")

import numpy as np

import concourse.bass as bass
import concourse.tile as tile
from concourse import bacc, mybir
from concourse import bass_utils

F32 = mybir.dt.float32
F32R = mybir.dt.float32r
I32 = mybir.dt.int32
AF = mybir.ActivationFunctionType
ALU = mybir.AluOpType
AX = mybir.AxisListType

NCORES = 8
B = 2
S = 512
D = 512
V = 32000
VP = 32768
VS = VP // NCORES          # 4096 vocab rows per core
NVT = VS // 128            # 32 vocab tiles per core
ND = D // 128              # 4 d-chunks
NST = S // 128             # 4 s-tiles
DFF = 2048
FFS = DFF // NCORES        # 256 ff per core
NFT = FFS // 128           # 2 ff tiles per core
NL = 4
EPS = 1e-5
RSQD = 1.0 / float(np.sqrt(np.float32(D)))
CLIP = 10.0 / RSQD         # clip bound on the unscaled scores
NEG = -1e30

RG = [list(range(NCORES))]

_cached = {}


def _build():
    nc = bacc.Bacc("TRN2", target_bir_lowering=False, debug=False,
                   enable_asserts=False, num_devices=NCORES)

    # ---------------- DRAM I/O ----------------
    d_wteT = nc.dram_tensor("wteT", [D, VS], F32R, kind="ExternalInput").ap()
    d_wte = nc.dram_tensor("wte_s", [VS, D], F32R, kind="ExternalInput").ap()
    d_wfull = nc.dram_tensor("wte_full", [V, D], F32, kind="ExternalInput").ap()
    d_wpeT = nc.dram_tensor("wpeT", [D, S + 1], F32R, kind="ExternalInput").ap()
    d_woT = nc.dram_tensor("w_oT", [D * 8, D], F32R, kind="ExternalInput").ap()
    d_w1T = nc.dram_tensor("w1T_s", [D, FFS], F32R, kind="ExternalInput").ap()
    d_w2T = nc.dram_tensor("w2T_s", [FFS, D], F32R, kind="ExternalInput").ap()
    d_lnm = nc.dram_tensor("ln_mlp", [D], F32, kind="ExternalInput").ap()
    d_lnf = nc.dram_tensor("ln_f", [D], F32, kind="ExternalInput").ap()
    d_x0 = nc.dram_tensor("x_b0", [S], I32, kind="ExternalInput").ap()
    d_x1 = nc.dram_tensor("x_b1", [S], I32, kind="ExternalInput").ap()
    d_mask = nc.dram_tensor("vmask", [VS], F32, kind="ExternalInput").ap()
    d_out = nc.dram_tensor("logits", [B, VS], F32, kind="ExternalOutput").ap()

    with tile.TileContext(nc) as tc:
        with tc.tile_pool(name="res", bufs=1) as res, \
             tc.tile_pool(name="wk", bufs=1) as wk, \
             tc.tile_pool(name="psum", bufs=1, space="PSUM") as psp, \
             tc.tile_pool(name="dram", bufs=1, space="DRAM") as dram:

            # ---------------- resident loads ----------------
            wteT = res.tile([128, ND, VS], F32R)
            nc.sync.dma_start(out=wteT[:], in_=d_wteT.rearrange("(c p) v -> p c v", p=128))
            w1T = res.tile([128, ND, FFS], F32R)
            nc.sync.dma_start(out=w1T[:], in_=d_w1T.rearrange("(c p) f -> p c f", p=128))
            w2T = res.tile([128, NFT, D], F32R)
            nc.sync.dma_start(out=w2T[:], in_=d_w2T.rearrange("(c p) d -> p c d", p=128))
            lnm = res.tile([128, ND], F32)
            nc.sync.dma_start(out=lnm[:], in_=d_lnm.rearrange("(c p) -> p c", p=128))
            lnf = res.tile([128, ND], F32)
            nc.sync.dma_start(out=lnf[:], in_=d_lnf.rearrange("(c p) -> p c", p=128))
            idx0 = res.tile([128, NST], I32)
            nc.sync.dma_start(out=idx0[:], in_=d_x0.rearrange("(c p) -> p c", p=128))
            idx1 = res.tile([128, NST], I32)
            nc.sync.dma_start(out=idx1[:], in_=d_x1.rearrange("(c p) -> p c", p=128))
            vmask = res.tile([128, NVT], F32)
            nc.sync.dma_start(out=vmask[:], in_=d_mask.rearrange("(v p) -> p v", p=128))

            from concourse.masks import make_identity
            ident_f = wk.tile([128, 128], F32, tag="ln_t1", bufs=2, name="ident_f")
            make_identity(nc, ident_f[:])
            ident = res.tile([128, 128], F32R)
            nc.vector.tensor_copy(ident[:], ident_f[:])
            ones_f = wk.tile([128, 2], F32, tag="cs_row", bufs=1, name="ones_f")
            nc.vector.memset(ones_f[:], 1.0)
            ones2 = res.tile([128, 2], F32R)
            nc.vector.tensor_copy(ones2[:], ones_f[:])
            ones_col = ones2[:, 0:1]
            ones_rf = wk.tile([1, 128], F32, tag="cs_row", bufs=1, name="ones_rf")
            nc.vector.memset(ones_rf[:], 1.0)
            ones_row = res.tile([1, 128], F32R)
            nc.vector.tensor_copy(ones_row[:], ones_rf[:])

            # w_o_effT[j, i] = sum_h w_oT[512h + j, i]
            weff = res.tile([128, ND, D], F32R)
            for h in range(8):
                blk = wk.tile([128, ND, D], F32R, tag="arbuf", bufs=2, name=f"woblk{h}")
                nc.sync.dma_start(
                    out=blk[:],
                    in_=d_woT[h * D:(h + 1) * D, :].rearrange("(c p) i -> p c i", p=128))
                if h == 0:
                    nc.vector.tensor_copy(weff[:], blk[:])
                else:
                    nc.vector.tensor_tensor(out=weff[:], in0=weff[:], in1=blk[:], op=ALU.add)

            # embeddings (gather) e[b]: [128, st, 512] (s-major)
            e_sb = []
            for b, idx in ((0, idx0), (1, idx1)):
                e_b = res.tile([128, NST, D], F32, name=f"e_{b}")
                for st in range(NST):
                    nc.gpsimd.indirect_dma_start(
                        out=e_b[:, st, :], out_offset=None,
                        in_=d_wfull[:, :],
                        in_offset=bass.IndirectOffsetOnAxis(ap=idx[:, st:st + 1], axis=0))
                e_sb.append(e_b)

            # f_kT master (T-major), float32r, zero-init
            zsrc = wk.tile([128, ND, S], F32, tag="arbuf", bufs=2, name="zsrc")
            nc.vector.memset(zsrc[:], 0.0)
            fT = []
            for b in range(B):
                f_b = res.tile([128, ND, S], F32R, name=f"fT_{b}")
                nc.vector.tensor_copy(f_b[:], zsrc[:])
                fT.append(f_b)

            # ---------------- krn2T prologue ----------------
            # scores rows 1..512 (queries), cols 0..511 (keys), causal mask,
            # clip +-10 (after 1/sqrt(d)), softmax over keys, fold 1/(t+1),
            # then transpose -> krn2T [s', t].
            krn2T = res.tile([128, NST, S], F32R)
            sc_ps = psp.tile([128, NST, S], F32, tag="acc4", bufs=1, name="sc_ps")
            for dc in range(ND):
                wpec = wk.tile([128, S + 1], F32R, tag="wpec", bufs=2, name=f"wpec{dc}")
                nc.sync.dma_start(out=wpec[:], in_=d_wpeT[dc * 128:(dc + 1) * 128, :])
                for tt in range(NST):
                    nc.tensor.matmul(
                        sc_ps[:, tt, :], lhsT=wpec[:, 1 + tt * 128:1 + tt * 128 + 128],
                        rhs=wpec[:, 0:S], start=(dc == 0), stop=(dc == ND - 1),
                        skip_group_check=True)
            for tt in range(NST):
                sc_sb = wk.tile([128, S], F32, tag="ln_t1", bufs=2, name="sc_sb")
                nc.vector.tensor_scalar(out=sc_sb[:], in0=sc_ps[:, tt, :],
                                        scalar1=CLIP, scalar2=-CLIP,
                                        op0=ALU.min, op1=ALU.max)
                sc_m = wk.tile([128, S], F32, tag="exw", bufs=1, name="sc_m")
                # keep where (tt*128 + p) - s' >= 0
                nc.gpsimd.affine_select(out=sc_m[:], in_=sc_sb[:],
                                        pattern=[[-1, S]], compare_op=ALU.is_ge,
                                        fill=NEG, base=tt * 128, channel_multiplier=1)
                ke = wk.tile([128, S], F32, tag="E", bufs=2, name="ke")
                krs = wk.tile([128, 1], F32, tag="rs", bufs=3, name="krs")
                nc.scalar.activation(ke[:], sc_m[:], AF.Exp, scale=RSQD, accum_out=krs[:])
                qi = wk.tile([128, 1], I32, tag="qi", bufs=2, name="qi")
                nc.gpsimd.iota(qi[:], pattern=[[0, 1]], base=tt * 128 + 1, channel_multiplier=1)
                qf = wk.tile([128, 1], F32, tag="qf", bufs=2, name="qf")
                nc.vector.tensor_copy(qf[:], qi[:])
                den_k = wk.tile([128, 1], F32, tag="rm", bufs=3, name="den_k")
                nc.vector.tensor_tensor(out=den_k[:], in0=krs[:], in1=qf[:], op=ALU.mult)
                rk = wk.tile([128, 1], F32, tag="rm2", bufs=3, name="rk")
                nc.vector.reciprocal(rk[:], den_k[:])
                k2 = wk.tile([128, S], F32R, tag="Es", bufs=2, name="k2")
                nc.vector.tensor_scalar(out=k2[:], in0=ke[:], scalar1=rk[:, :1],
                                        scalar2=None, op0=ALU.mult)
                for st in range(NST):
                    tr_ps = psp.tile([128, 128], F32R, tag="tr", bufs=2)
                    nc.tensor.transpose(tr_ps[:], k2[:, st * 128:(st + 1) * 128], ident[:])
                    nc.vector.tensor_copy(krn2T[:, st, tt * 128:(tt + 1) * 128], tr_ps[:])

            # ---------------- layer-0 colsum ----------------
            cs = wk.tile([128, ND], F32, tag="cs")
            for dc in range(ND):
                nc.vector.tensor_reduce(out=cs[:, dc:dc + 1], in_=wteT[:, dc, :],
                                        axis=AX.X, op=ALU.add)
            cs_in = dram.tile([D], F32, tag="cs_in")
            cs_out = dram.tile([D], F32, tag="cs_out", addr_space="Shared")
            nc.sync.dma_start(out=cs_in[:].rearrange("(c p) -> p c", p=128), in_=cs[:])
            nc.gpsimd.collective_compute("AllReduce", ALU.add, replica_groups=RG,
                                         ins=[cs_in[:]], outs=[cs_out[:]])
            cs_row = wk.tile([1, D], F32, tag="cs_row")
            nc.sync.dma_start(out=cs_row[:], in_=cs_out[:].rearrange("(a s) -> a s", a=1))
            ex0_row = wk.tile([1, D], F32R, tag="ex0_row")
            nc.vector.tensor_scalar(out=ex0_row[:], in0=cs_row[:], scalar1=1.0 / V,
                                    scalar2=None, op0=ALU.mult)

            # ---------------- helpers ----------------
            def ln_apply(b, ln_col, out_tile):
                """out_tile [128, ND, S] f32r = layernorm(fT[b]) * ln_col (T-major)."""
                sq = wk.tile([128, ND, S], F32R, tag="delta_sb", bufs=1, name="sq")
                nc.scalar.activation(sq[:], fT[b][:], AF.Square)
                s1 = psp.tile([1, S], F32, tag="zp", bufs=2, name="s1")
                s2 = psp.tile([1, S], F32, tag="zp", bufs=2, name="s2")
                for dc in range(ND):
                    nc.tensor.matmul(s1[:], lhsT=ones_col, rhs=fT[b][:, dc, :],
                                     start=(dc == 0), stop=(dc == ND - 1))
                for dc in range(ND):
                    nc.tensor.matmul(s2[:], lhsT=ones_col, rhs=sq[:, dc, :],
                                     start=(dc == 0), stop=(dc == ND - 1))
                m_row = wk.tile([1, S], F32R, tag="m_row", bufs=1)
                nc.vector.tensor_scalar(out=m_row[:], in0=s1[:], scalar1=1.0 / D,
                                        scalar2=None, op0=ALU.mult)
                v_row = wk.tile([1, S], F32, tag="v_row", bufs=1)
                nc.vector.tensor_scalar(out=v_row[:], in0=s2[:], scalar1=1.0 / D,
                                        scalar2=EPS, op0=ALU.mult, op1=ALU.add)
                sd_row = wk.tile([1, S], F32, tag="sd_row", bufs=1)
                nc.vector.tensor_tensor(out=sd_row[:], in0=m_row[:], in1=m_row[:], op=ALU.mult)
                nc.vector.tensor_tensor(out=v_row[:], in0=v_row[:], in1=sd_row[:], op=ALU.subtract)
                nc.scalar.activation(sd_row[:], v_row[:], AF.Sqrt)
                rstd_row = wk.tile([1, S], F32R, tag="rstd_row", bufs=1)
                with nc.allow_low_precision(reason="rstd rounded to f32r for matmul rhs"):
                    nc.vector.reciprocal(rstd_row[:], sd_row[:])
                mb = psp.tile([128, S], F32, tag="tr", bufs=2, name="mb")
                rb = psp.tile([128, S], F32, tag="tr", bufs=2, name="rb")
                nc.tensor.matmul(mb[:], lhsT=ones_row[:], rhs=m_row[:], start=True, stop=True)
                nc.tensor.matmul(rb[:], lhsT=ones_row[:], rhs=rstd_row[:], start=True, stop=True)
                for dc in range(ND):
                    t1 = wk.tile([128, S], F32, tag="ln_t1", bufs=2)
                    nc.vector.tensor_tensor(out=t1[:], in0=fT[b][:, dc, :], in1=mb[:], op=ALU.subtract)
                    nc.vector.tensor_tensor(out=t1[:], in0=t1[:], in1=rb[:], op=ALU.mult)
                    nc.vector.tensor_scalar(out=out_tile[:, dc, :], in0=t1[:],
                                            scalar1=ln_col[:, dc:dc + 1], scalar2=None,
                                            op0=ALU.mult)

            def sm_tail(b, k, V_sb):
                """delta = krn2 @ V (s-major) -> transpose -> deltaW^T -> fT += ;
                then LN + sharded MLP + AllReduce + fT +=."""
                delta_ps = psp.tile([128, NST, S], F32, tag="acc4", bufs=1, name="delta_ps")
                for tt in range(NST):
                    for st in range(NST):
                        nc.tensor.matmul(
                            delta_ps[:, tt, :],
                            lhsT=krn2T[:, st, tt * 128:(tt + 1) * 128],
                            rhs=V_sb[:, st, :],
                            start=(st == 0), stop=(st == NST - 1))
                delta_sb = wk.tile([128, NST, S], F32R, tag="delta_sb", bufs=1)
                for tt in range(NST):
                    nc.vector.tensor_copy(delta_sb[:, tt, :], delta_ps[:, tt, :])
                deltaT = wk.tile([128, ND, S], F32R, tag="deltaT", bufs=1)
                for tt in range(NST):
                    for dc in range(ND):
                        tr_ps = psp.tile([128, 128], F32R, tag="tr", bufs=2)
                        nc.tensor.transpose(tr_ps[:], delta_sb[:, tt, dc * 128:(dc + 1) * 128],
                                            ident[:])
                        nc.vector.tensor_copy(deltaT[:, dc, tt * 128:(tt + 1) * 128], tr_ps[:])
                dW_ps = psp.tile([128, ND, S], F32, tag="acc4", bufs=1, name="dW_ps")
                for it in range(ND):
                    for jc in range(ND):
                        nc.tensor.matmul(
                            dW_ps[:, it, :],
                            lhsT=weff[:, jc, it * 128:(it + 1) * 128],
                            rhs=deltaT[:, jc, :],
                            start=(jc == 0), stop=(jc == ND - 1))
                for it in range(ND):
                    nc.vector.tensor_tensor(out=fT[b][:, it, :], in0=fT[b][:, it, :],
                                            in1=dW_ps[:, it, :], op=ALU.add)
                # LN + MLP (sharded over d_ff)
                h_sb = wk.tile([128, ND, S], F32R, tag="deltaT", bufs=1, name="h_sb")
                ln_apply(b, lnm, h_sb)
                gT = wk.tile([128, NFT, S], F32R, tag="gT", bufs=1)
                for ff in range(NFT):
                    g_ps = psp.tile([128, S], F32, tag="zp", bufs=2, name="g_ps")
                    for dc in range(ND):
                        nc.tensor.matmul(g_ps[:], lhsT=w1T[:, dc, ff * 128:(ff + 1) * 128],
                                         rhs=h_sb[:, dc, :],
                                         start=(dc == 0), stop=(dc == ND - 1))
                    nc.scalar.activation(gT[:, ff, :], g_ps[:], AF.Gelu)
                mlp_ps = psp.tile([128, ND, S], F32, tag="acc4", bufs=1, name="mlp_ps")
                for dc in range(ND):
                    for ff in range(NFT):
                        nc.tensor.matmul(mlp_ps[:, dc, :],
                                         lhsT=w2T[:, ff, dc * 128:(dc + 1) * 128],
                                         rhs=gT[:, ff, :],
                                         start=(ff == 0), stop=(ff == NFT - 1))
                mlp_sb = wk.tile([128, ND, S], F32, tag="arbuf", bufs=2, name="mlp_sb")
                for dc in range(ND):
                    nc.vector.tensor_copy(mlp_sb[:, dc, :], mlp_ps[:, dc, :])
                m_in = dram.tile([ND * 128 * S], F32, tag="m_in", bufs=2)
                m_out = dram.tile([ND * 128 * S], F32, tag="m_out", bufs=2, addr_space="Shared")
                nc.sync.dma_start(out=m_in[:].rearrange("(c p s) -> p c s", p=128, s=S),
                                  in_=mlp_sb[:])
                nc.gpsimd.collective_compute("AllReduce", ALU.add, replica_groups=RG,
                                             ins=[m_in[:]], outs=[m_out[:]])
                mlp2 = wk.tile([128, ND, S], F32, tag="arbuf", bufs=2, name="mlp2")
                nc.sync.dma_start(out=mlp2[:],
                                  in_=m_out[:].rearrange("(c p s) -> p c s", p=128, s=S))
                for dc in range(ND):
                    nc.vector.tensor_tensor(out=fT[b][:, dc, :], in0=fT[b][:, dc, :],
                                            in1=mlp2[:, dc, :], op=ALU.add)

            # -------- layer 0 (f=0: R uniform, ex_wte = colmean(wte)) --------
            for b in range(B):
                exb_ps = psp.tile([128, S], F32, tag="tr", bufs=2, name="exb_ps")
                nc.tensor.matmul(exb_ps[:], lhsT=ones_row[:], rhs=ex0_row[:],
                                 start=True, stop=True)
                V_sb = wk.tile([128, NST, D], F32R, tag="V", bufs=1, name="V0")
                for st in range(NST):
                    nc.vector.tensor_tensor(out=V_sb[:, st, :], in0=e_sb[b][:, st, :],
                                            in1=exb_ps[:], op=ALU.subtract)
                sm_tail(b, 0, V_sb)

            # ---------------- layers 1..3 ----------------
            for k in range(1, NL):
                nd_out = {}
                for b in range(B):
                    num_ps = psp.tile([128, NST, D], F32, tag="acc4", bufs=1, name="num_ps")
                    den_ps = psp.tile([128, NST, 2], F32, tag="tr", bufs=2, name="den_ps")
                    for v in range(NVT):
                        wt = wk.tile([128, D], F32R, tag="wt", bufs=3)
                        nc.sync.dma_start(out=wt[:], in_=d_wte[v * 128:(v + 1) * 128, :])
                        z_ps = psp.tile([128, S], F32, tag="zp", bufs=2, name="z_ps")
                        for dc in range(ND):
                            nc.tensor.matmul(z_ps[:], lhsT=wteT[:, dc, v * 128:(v + 1) * 128],
                                             rhs=fT[b][:, dc, :],
                                             start=(dc == 0), stop=(dc == ND - 1))
                        nmax = wk.tile([128, 1], F32, tag="nmax", bufs=3)
                        nc.vector.tensor_reduce(out=nmax[:], in_=z_ps[:], axis=AX.X,
                                                op=ALU.max, negate=True)
                        E = wk.tile([128, S], F32R, tag="E", bufs=2)
                        rs = wk.tile([128, 1], F32, tag="rs", bufs=3)
                        nc.scalar.activation(E[:], z_ps[:], AF.Exp, bias=nmax[:, :1],
                                             accum_out=rs[:])
                        rm = wk.tile([128, 1], F32, tag="rm", bufs=3)
                        nc.vector.reciprocal(rm[:], rs[:])
                        rm2 = wk.tile([128, 1], F32, tag="rm2", bufs=3)
                        nc.vector.tensor_tensor(out=rm2[:], in0=rm[:],
                                                in1=vmask[:, v:v + 1], op=ALU.mult)
                        Es = wk.tile([128, S], F32R, tag="Es", bufs=2)
                        nc.vector.tensor_scalar(out=Es[:], in0=E[:], scalar1=rm2[:, :1],
                                                scalar2=None, op0=ALU.mult)
                        first, last = (v == 0), (v == NVT - 1)
                        for st in range(NST):
                            nc.tensor.matmul(num_ps[:, st, :],
                                             lhsT=Es[:, st * 128:(st + 1) * 128],
                                             rhs=wt[:], start=first, stop=last,
                                             skip_group_check=True)
                            nc.tensor.matmul(den_ps[:, st, :],
                                             lhsT=Es[:, st * 128:(st + 1) * 128],
                                             rhs=ones2[:], start=first, stop=last,
                                             skip_group_check=True)
                    nsb = wk.tile([128, NST, D], F32, tag="arbuf", bufs=2, name="nsb")
                    for st in range(NST):
                        nc.vector.tensor_copy(nsb[:, st, :], num_ps[:, st, :])
                    dsb = wk.tile([128, NST], F32, tag="dsb", bufs=2)
                    nc.vector.tensor_copy(dsb[:], den_ps[:, :, 0])
                    nd_in = dram.tile([NST * 128 * D + S], F32, tag="nd_in", bufs=2)
                    ndo = dram.tile([NST * 128 * D + S], F32, tag="nd_out", bufs=2,
                                    addr_space="Shared")
                    nc.sync.dma_start(
                        out=nd_in[:NST * 128 * D].rearrange("(c p s) -> p c s", p=128, s=D),
                        in_=nsb[:])
                    nc.sync.dma_start(
                        out=nd_in[NST * 128 * D:].rearrange("(c p) -> p c", p=128),
                        in_=dsb[:])
                    nc.gpsimd.collective_compute("AllReduce", ALU.add, replica_groups=RG,
                                                 ins=[nd_in[:]], outs=[ndo[:]])
                    nd_out[b] = ndo
                for b in range(B):
                    ndo = nd_out[b]
                    n2 = wk.tile([128, NST, D], F32, tag="arbuf", bufs=2, name="n2")
                    nc.sync.dma_start(
                        out=n2[:],
                        in_=ndo[:NST * 128 * D].rearrange("(c p s) -> p c s", p=128, s=D))
                    d2 = wk.tile([128, NST], F32, tag="d2", bufs=2)
                    nc.sync.dma_start(
                        out=d2[:], in_=ndo[NST * 128 * D:].rearrange("(c p) -> p c", p=128))
                    rden = wk.tile([128, NST], F32, tag="rden", bufs=2)
                    nc.vector.reciprocal(rden[:], d2[:])
                    V_sb = wk.tile([128, NST, D], F32R, tag="V", bufs=1, name="V_sb")
                    for st in range(NST):
                        exw = wk.tile([128, D], F32, tag="exw", bufs=1)
                        nc.vector.tensor_scalar(out=exw[:], in0=n2[:, st, :],
                                                scalar1=rden[:, st:st + 1], scalar2=None,
                                                op0=ALU.mult)
                        nc.vector.tensor_tensor(out=V_sb[:, st, :], in0=e_sb[b][:, st, :],
                                                in1=exw[:], op=ALU.subtract)
                    sm_tail(b, k, V_sb)

            # ---------------- final LN + logits ----------------
            xl = wk.tile([128, ND, B], F32R, tag="xl")
            for b in range(B):
                xn = wk.tile([128, ND, S], F32R, tag="V", bufs=1, name="xn")
                ln_apply(b, lnf, xn)
                for dc in range(ND):
                    nc.vector.tensor_copy(xl[:, dc, b:b + 1], xn[:, dc, S - 1:S])
            for v8 in range(VS // 512):
                lg_ps = psp.tile([B, 512], F32, tag="zp", bufs=2, name="lg_ps")
                for dc in range(ND):
                    nc.tensor.matmul(lg_ps[:], lhsT=xl[:, dc, :],
                                     rhs=wteT[:, dc, v8 * 512:(v8 + 1) * 512],
                                     start=(dc == 0), stop=(dc == ND - 1))
                lg_sb = wk.tile([B, 512], F32, tag="lgsb", bufs=2)
                nc.vector.tensor_copy(lg_sb[:], lg_ps[:])
                nc.sync.dma_start(out=d_out[:, v8 * 512:(v8 + 1) * 512], in_=lg_sb[:])

    nc.compile()
    return nc


def _prep_inputs(x, wte, wpe, w_o, w1, w2, ln_mlp_w, ln_f_w):
    """Host-side sharding / layout prep (pure slicing, padding, transposes)."""
    x = np.asarray(x, dtype=np.int32)
    wte = np.ascontiguousarray(np.asarray(wte, dtype=np.float32))
    wpe = np.asarray(wpe, dtype=np.float32)
    w_o = np.asarray(w_o, dtype=np.float32)
    w1 = np.asarray(w1, dtype=np.float32)
    w2 = np.asarray(w2, dtype=np.float32)
    ln_mlp_w = np.asarray(ln_mlp_w, dtype=np.float32)
    ln_f_w = np.asarray(ln_f_w, dtype=np.float32)

    wte_pad = np.zeros((VP, D), dtype=np.float32)
    wte_pad[:V] = wte
    wteT_pad = np.ascontiguousarray(wte_pad.T)          # [D, VP]
    wpeT = np.ascontiguousarray(wpe[:S + 1].T)          # [D, S+1]
    w_oT = np.ascontiguousarray(w_o.T)                  # [8D, D]
    w1T = np.ascontiguousarray(w1.T)                    # [D, DFF]
    w2T = np.ascontiguousarray(w2.T)                    # [DFF, D]

    in_maps = []
    for c in range(NCORES):
        v0, v1 = c * VS, (c + 1) * VS
        f0, f1 = c * FFS, (c + 1) * FFS
        mask = np.zeros(VS, dtype=np.float32)
        nreal = min(max(V - v0, 0), VS)
        mask[:nreal] = 1.0
        in_maps.append({
            "wteT": np.ascontiguousarray(wteT_pad[:, v0:v1]),
            "wte_s": np.ascontiguousarray(wte_pad[v0:v1]),
            "wte_full": wte,
            "wpeT": wpeT,
            "w_oT": w_oT,
            "w1T_s": np.ascontiguousarray(w1T[:, f0:f1]),
            "w2T_s": np.ascontiguousarray(w2T[f0:f1]),
            "ln_mlp": ln_mlp_w,
            "ln_f": ln_f_w,
            "x_b0": np.ascontiguousarray(x[0]),
            "x_b1": np.ascontiguousarray(x[1]),
            "vmask": mask,
        })
    return in_maps


def kernel(x, wte, wpe, w_o, w1, w2, ln_mlp_w, ln_f_w, _trace=False):
    if "nc" not in _cached:
        _cached["nc"] = _build()
    nc = _cached["nc"]
    in_maps = _prep_inputs(x, wte, wpe, w_o, w1, w2, ln_mlp_w, ln_f_w)
    res = bass_utils.run_bass_kernel_spmd(nc, in_maps, core_ids=list(range(NCORES)),
                                          trace=_trace)
    _cached["last_result"] = res
    logits = np.concatenate([r["logits"] for r in res.results], axis=1)  # [B, VP]
    return logits[:, None, :V].astype(np.float32)


# revision 16
# speedup vs baseline: 1.4539x; 1.4539x over previous
"""Trainium2 Bass kernel for nn_CausalGDM (dense transformer with
vocab-projection softmax attention), 8-way vocab-sharded across 8 NeuronCores.

Sharding: vocab V=32000 padded to 32768, split 8 ways (4096 rows/core).
Each core processes BOTH batches (batch-pipelined to hide all-reduces).
MLP is sharded 8 ways over d_ff. num/den of the gd_step and the MLP partial
sums are combined with AllReduce.

All matmuls run as float32r (TF32-like, full PE rate at N>=256).

SBUF plan (KB/partition, budget ~208):
  res: wteT 64, w1T 4, w2T 4, weff 8, e 16, fT 16, krn2T 8, misc ~1.3 = ~121
  wk tags: arbuf 8, wpec 4, ln_t1 4, exw 2, E 4, Es 4, rows 8, delta_sb 8,
           deltaT/h 8, gT 4, V/xn 8, wt 6, small ~1 = ~70
PSUM plan (banks, budget 8): acc4 (num/delta/dW/mlp/scores) 4,
  zp (Z/g/rows/logits) 2, tr (transp/den/broadcasts) 2.
"""

import sys

sys.path.insert(0, "/opt/trn_rl_repo")

import numpy as np

import concourse.bass as bass
import concourse.tile as tile
from concourse import bacc, mybir
from concourse import bass_utils

F32 = mybir.dt.float32
F32R = mybir.dt.float32r
I32 = mybir.dt.int32
AF = mybir.ActivationFunctionType
ALU = mybir.AluOpType
AX = mybir.AxisListType

NCORES = 8
B = 2
S = 512
D = 512
V = 32000
VP = 32768
VS = VP // NCORES          # 4096 vocab rows per core
NVT = VS // 128            # 32 vocab tiles per core
ND = D // 128              # 4 d-chunks
NST = S // 128             # 4 s-tiles
DFF = 2048
FFS = DFF // NCORES        # 256 ff per core
NFT = FFS // 128           # 2 ff tiles per core
NL = 4
EPS = 1e-5
RSQD = 1.0 / float(np.sqrt(np.float32(D)))
CLIP = 10.0 / RSQD         # clip bound on the unscaled scores
NEG = -1e30

RG = [list(range(NCORES))]

_cached = {}


def _build(n_devices=NCORES, skip_ar=False):
    nc = bacc.Bacc("TRN2", target_bir_lowering=False, debug=False,
                   enable_asserts=False, num_devices=n_devices)

    def all_reduce(nc_, ins, outs):
        if skip_ar:
            nc_.sync.dma_start(out=outs[0], in_=ins[0])
        else:
            nc_.gpsimd.collective_compute("AllReduce", ALU.add, replica_groups=RG,
                                          ins=ins, outs=outs)

    # ---------------- DRAM I/O ----------------
    d_wteT = nc.dram_tensor("wteT", [D, VS], F32R, kind="ExternalInput").ap()
    d_wte = nc.dram_tensor("wte_s", [VS, D], F32R, kind="ExternalInput").ap()
    d_wfull = nc.dram_tensor("wte_full", [V, D], F32, kind="ExternalInput").ap()
    d_wpeT = nc.dram_tensor("wpeT", [D, S + 1], F32R, kind="ExternalInput").ap()
    d_woT = nc.dram_tensor("w_oT", [D * 8, D], F32R, kind="ExternalInput").ap()
    d_w1T = nc.dram_tensor("w1T_s", [D, FFS], F32R, kind="ExternalInput").ap()
    d_w2T = nc.dram_tensor("w2T_s", [FFS, D], F32R, kind="ExternalInput").ap()
    d_lnm = nc.dram_tensor("ln_mlp", [D], F32, kind="ExternalInput").ap()
    d_lnf = nc.dram_tensor("ln_f", [D], F32, kind="ExternalInput").ap()
    d_x0 = nc.dram_tensor("x_b0", [S], I32, kind="ExternalInput").ap()
    d_x1 = nc.dram_tensor("x_b1", [S], I32, kind="ExternalInput").ap()
    d_mask = nc.dram_tensor("vmask", [VS], F32, kind="ExternalInput").ap()
    d_out = nc.dram_tensor("logits", [B, VS], F32, kind="ExternalOutput").ap()

    with tile.TileContext(nc) as tc:
        with tc.tile_pool(name="res", bufs=1) as res, \
             tc.tile_pool(name="wk", bufs=1) as wk, \
             tc.tile_pool(name="psum", bufs=1, space="PSUM") as psp, \
             tc.tile_pool(name="dram", bufs=1, space="DRAM") as dram:

            # ---------------- resident loads ----------------
            wteT = res.tile([128, ND, VS], F32R)
            nc.sync.dma_start(out=wteT[:], in_=d_wteT.rearrange("(c p) v -> p c v", p=128))
            w1T = res.tile([128, ND, FFS], F32R)
            nc.sync.dma_start(out=w1T[:], in_=d_w1T.rearrange("(c p) f -> p c f", p=128))
            w2T = res.tile([128, NFT, D], F32R)
            nc.sync.dma_start(out=w2T[:], in_=d_w2T.rearrange("(c p) d -> p c d", p=128))
            lnm = res.tile([128, ND], F32)
            nc.sync.dma_start(out=lnm[:], in_=d_lnm.rearrange("(c p) -> p c", p=128))
            lnf = res.tile([128, ND], F32)
            nc.sync.dma_start(out=lnf[:], in_=d_lnf.rearrange("(c p) -> p c", p=128))
            idx0 = res.tile([128, NST], I32)
            nc.sync.dma_start(out=idx0[:], in_=d_x0.rearrange("(c p) -> p c", p=128))
            idx1 = res.tile([128, NST], I32)
            nc.sync.dma_start(out=idx1[:], in_=d_x1.rearrange("(c p) -> p c", p=128))
            vmask = res.tile([128, NVT], F32)
            nc.sync.dma_start(out=vmask[:], in_=d_mask.rearrange("(v p) -> p v", p=128))

            from concourse.masks import make_identity
            ident = res.tile([128, 128], F32)
            make_identity(nc, ident[:])
            ones_f = wk.tile([128, 2], F32, tag="cs_row", bufs=1, name="ones_f")
            nc.vector.memset(ones_f[:], 1.0)
            ones2 = res.tile([128, 2], F32R)
            nc.vector.tensor_copy(ones2[:], ones_f[:])
            ones_col = ones2[:, 0:1]
            ones_rf = wk.tile([1, 128], F32, tag="cs_row", bufs=1, name="ones_rf")
            nc.vector.memset(ones_rf[:], 1.0)
            ones_row = res.tile([1, 128], F32R)
            nc.vector.tensor_copy(ones_row[:], ones_rf[:])

            # w_o_effT[j, i] = sum_h w_oT[512h + j, i]
            weff = res.tile([128, ND, D], F32R)
            for h in range(8):
                blk = wk.tile([128, ND, D], F32R, tag="arbuf", bufs=2, name=f"woblk{h}")
                nc.sync.dma_start(
                    out=blk[:],
                    in_=d_woT[h * D:(h + 1) * D, :].rearrange("(c p) i -> p c i", p=128))
                if h == 0:
                    nc.vector.tensor_copy(weff[:], blk[:])
                else:
                    nc.vector.tensor_tensor(out=weff[:], in0=weff[:], in1=blk[:], op=ALU.add)

            # ---------------- krn2T prologue ----------------
            # scores rows 1..512 (queries), cols 0..511 (keys), causal mask,
            # clip +-10 (after 1/sqrt(d)), softmax over keys, fold 1/(t+1),
            # then transpose -> krn2T [s', t].
            krn2T = res.tile([128, NST, S], F32R)
            sc_ps = psp.tile([128, NST, S], F32, tag="acc4", bufs=1, name="sc_ps")
            for dc in range(ND):
                wpec = wk.tile([128, S + 1], F32R, tag="wpec", bufs=2, name=f"wpec{dc}")
                nc.sync.dma_start(out=wpec[:], in_=d_wpeT[dc * 128:(dc + 1) * 128, :])
                for tt in range(NST):
                    nc.tensor.matmul(
                        sc_ps[:, tt, :], lhsT=wpec[:, 1 + tt * 128:1 + tt * 128 + 128],
                        rhs=wpec[:, 0:S], start=(dc == 0), stop=(dc == ND - 1),
                        skip_group_check=True)
            for tt in range(NST):
                sc_sb = wk.tile([128, S], F32, tag="ln_t1", bufs=1, name="sc_sb")
                nc.vector.tensor_scalar(out=sc_sb[:], in0=sc_ps[:, tt, :],
                                        scalar1=CLIP, scalar2=-CLIP,
                                        op0=ALU.min, op1=ALU.max)
                sc_m = wk.tile([128, S], F32, tag="Es", bufs=2, name="sc_m")
                # keep where (tt*128 + p) - s' >= 0
                nc.gpsimd.affine_select(out=sc_m[:], in_=sc_sb[:],
                                        pattern=[[-1, S]], compare_op=ALU.is_ge,
                                        fill=NEG, base=tt * 128, channel_multiplier=1)
                ke = wk.tile([128, S], F32, tag="E", bufs=2, name="ke")
                krs = wk.tile([128, 1], F32, tag="rs", bufs=3, name="krs")
                nc.scalar.activation(ke[:], sc_m[:], AF.Exp, scale=RSQD, accum_out=krs[:])
                qi = wk.tile([128, 1], I32, tag="qi", bufs=2, name="qi")
                nc.gpsimd.iota(qi[:], pattern=[[0, 1]], base=tt * 128 + 1, channel_multiplier=1)
                qf = wk.tile([128, 1], F32, tag="qf", bufs=2, name="qf")
                nc.vector.tensor_copy(qf[:], qi[:])
                den_k = wk.tile([128, 1], F32, tag="rm", bufs=3, name="den_k")
                nc.vector.tensor_tensor(out=den_k[:], in0=krs[:], in1=qf[:], op=ALU.mult)
                rk = wk.tile([128, 1], F32, tag="rm2", bufs=3, name="rk")
                nc.vector.reciprocal(rk[:], den_k[:])
                k2 = wk.tile([128, S], F32, tag="Es", bufs=2, name="k2")
                nc.vector.tensor_scalar(out=k2[:], in0=ke[:], scalar1=rk[:, :1],
                                        scalar2=None, op0=ALU.mult)
                for st in range(NST):
                    tr_ps = psp.tile([128, 128], F32, tag="tr", bufs=2)
                    nc.tensor.transpose(tr_ps[:], k2[:, st * 128:(st + 1) * 128], ident[:])
                    nc.vector.tensor_copy(krn2T[:, st, tt * 128:(tt + 1) * 128], tr_ps[:])

            # embeddings (gather) e[b]: [128, st, 512] (s-major)
            e_sb = []
            for b, idx in ((0, idx0), (1, idx1)):
                e_b = res.tile([128, NST, D], F32, name=f"e_{b}")
                for st in range(NST):
                    nc.gpsimd.indirect_dma_start(
                        out=e_b[:, st, :], out_offset=None,
                        in_=d_wfull[:, :],
                        in_offset=bass.IndirectOffsetOnAxis(ap=idx[:, st:st + 1], axis=0))
                e_sb.append(e_b)

            # f_kT master (T-major), float32r, zero-init
            zsrc = wk.tile([128, ND, S], F32, tag="arbuf", bufs=2, name="zsrc")
            nc.vector.memset(zsrc[:], 0.0)
            fT = []
            for b in range(B):
                f_b = res.tile([128, ND, S], F32R, name=f"fT_{b}")
                nc.vector.tensor_copy(f_b[:], zsrc[:])
                fT.append(f_b)

            # ---------------- layer-0 colsum ----------------
            cs = wk.tile([128, ND], F32, tag="cs")
            for dc in range(ND):
                nc.vector.tensor_reduce(out=cs[:, dc:dc + 1], in_=wteT[:, dc, :],
                                        axis=AX.X, op=ALU.add)
            cs_in = dram.tile([D], F32, tag="cs_in")
            cs_out = dram.tile([D], F32, tag="cs_out", addr_space="Shared")
            nc.sync.dma_start(out=cs_in[:].rearrange("(c p) -> p c", p=128), in_=cs[:])
            all_reduce(nc, [cs_in[:]], [cs_out[:]])
            cs_row = wk.tile([1, D], F32, tag="cs_row")
            nc.sync.dma_start(out=cs_row[:], in_=cs_out[:].rearrange("(a s) -> a s", a=1))
            ex0_row = wk.tile([1, D], F32R, tag="ex0_row")
            nc.vector.tensor_scalar(out=ex0_row[:], in0=cs_row[:], scalar1=1.0 / V,
                                    scalar2=None, op0=ALU.mult)

            # ---------------- helpers ----------------
            def ln_apply(b, ln_col, out_tile):
                """out_tile [128, ND, S] f32r = layernorm(fT[b]) * ln_col (T-major)."""
                sq = wk.tile([128, ND, S], F32R, tag="delta_sb", bufs=1, name="sq")
                nc.scalar.activation(sq[:], fT[b][:], AF.Square)
                s1 = psp.tile([1, S], F32, tag="zp", bufs=2, name="s1")
                s2 = psp.tile([1, S], F32, tag="zp", bufs=2, name="s2")
                for dc in range(ND):
                    nc.tensor.matmul(s1[:], lhsT=ones_col, rhs=fT[b][:, dc, :],
                                     start=(dc == 0), stop=(dc == ND - 1))
                for dc in range(ND):
                    nc.tensor.matmul(s2[:], lhsT=ones_col, rhs=sq[:, dc, :],
                                     start=(dc == 0), stop=(dc == ND - 1))
                m_row = wk.tile([1, S], F32R, tag="m_row", bufs=1)
                nc.vector.tensor_scalar(out=m_row[:], in0=s1[:], scalar1=1.0 / D,
                                        scalar2=None, op0=ALU.mult)
                v_row = wk.tile([1, S], F32, tag="v_row", bufs=1)
                nc.vector.tensor_scalar(out=v_row[:], in0=s2[:], scalar1=1.0 / D,
                                        scalar2=EPS, op0=ALU.mult, op1=ALU.add)
                sd_row = wk.tile([1, S], F32, tag="sd_row", bufs=1)
                nc.vector.tensor_tensor(out=sd_row[:], in0=m_row[:], in1=m_row[:], op=ALU.mult)
                nc.vector.tensor_tensor(out=v_row[:], in0=v_row[:], in1=sd_row[:], op=ALU.subtract)
                nc.scalar.activation(sd_row[:], v_row[:], AF.Sqrt)
                rstd_row = wk.tile([1, S], F32R, tag="rstd_row", bufs=1)
                with nc.allow_low_precision(reason="rstd rounded to f32r for matmul rhs"):
                    nc.vector.reciprocal(rstd_row[:], sd_row[:])
                mb = psp.tile([128, S], F32, tag="tr", bufs=2, name="mb")
                rb = psp.tile([128, S], F32, tag="tr", bufs=2, name="rb")
                nc.tensor.matmul(mb[:], lhsT=ones_row[:], rhs=m_row[:], start=True, stop=True)
                nc.tensor.matmul(rb[:], lhsT=ones_row[:], rhs=rstd_row[:], start=True, stop=True)
                for dc in range(ND):
                    t1 = wk.tile([128, S], F32, tag="ln_t1", bufs=2)
                    nc.vector.tensor_tensor(out=t1[:], in0=fT[b][:, dc, :], in1=mb[:], op=ALU.subtract)
                    nc.vector.tensor_tensor(out=t1[:], in0=t1[:], in1=rb[:], op=ALU.mult)
                    nc.vector.tensor_scalar(out=out_tile[:, dc, :], in0=t1[:],
                                            scalar1=ln_col[:, dc:dc + 1], scalar2=None,
                                            op0=ALU.mult)

            def sm_tail(b, k, V_sb):
                """delta = krn2 @ V (s-major) -> transpose -> deltaW^T -> fT += ;
                then LN + sharded MLP + AllReduce + fT +=."""
                delta_ps = psp.tile([128, NST, S], F32, tag="acc4", bufs=1, name="delta_ps")
                for tt in range(NST):
                    for st in range(NST):
                        nc.tensor.matmul(
                            delta_ps[:, tt, :],
                            lhsT=krn2T[:, st, tt * 128:(tt + 1) * 128],
                            rhs=V_sb[:, st, :],
                            start=(st == 0), stop=(st == NST - 1))
                delta_sb = wk.tile([128, NST, S], F32, tag="delta_sb", bufs=1)
                for tt in range(NST):
                    nc.vector.tensor_copy(delta_sb[:, tt, :], delta_ps[:, tt, :])
                deltaT = wk.tile([128, ND, S], F32R, tag="deltaT", bufs=1)
                for dc in range(ND):
                    trb_ps = psp.tile([128, S], F32, tag="tr", bufs=2, name="trb_ps")
                    for tt in range(NST):
                        nc.tensor.transpose(trb_ps[:, tt * 128:(tt + 1) * 128],
                                            delta_sb[:, tt, dc * 128:(dc + 1) * 128],
                                            ident[:])
                    nc.vector.tensor_copy(deltaT[:, dc, :], trb_ps[:])
                dW_ps = psp.tile([128, ND, S], F32, tag="acc4", bufs=1, name="dW_ps")
                for it in range(ND):
                    for jc in range(ND):
                        nc.tensor.matmul(
                            dW_ps[:, it, :],
                            lhsT=weff[:, jc, it * 128:(it + 1) * 128],
                            rhs=deltaT[:, jc, :],
                            start=(jc == 0), stop=(jc == ND - 1))
                for it in range(ND):
                    nc.vector.tensor_tensor(out=fT[b][:, it, :], in0=fT[b][:, it, :],
                                            in1=dW_ps[:, it, :], op=ALU.add)
                # LN + MLP (sharded over d_ff)
                h_sb = wk.tile([128, ND, S], F32R, tag="deltaT", bufs=1, name="h_sb")
                ln_apply(b, lnm, h_sb)
                gT = wk.tile([128, NFT, S], F32R, tag="gT", bufs=1)
                for ff in range(NFT):
                    g_ps = psp.tile([128, S], F32, tag="zp", bufs=2, name="g_ps")
                    for dc in range(ND):
                        nc.tensor.matmul(g_ps[:], lhsT=w1T[:, dc, ff * 128:(ff + 1) * 128],
                                         rhs=h_sb[:, dc, :],
                                         start=(dc == 0), stop=(dc == ND - 1))
                    nc.scalar.activation(gT[:, ff, :], g_ps[:], AF.Gelu)
                mlp_ps = psp.tile([128, ND, S], F32, tag="acc4", bufs=1, name="mlp_ps")
                for dc in range(ND):
                    for ff in range(NFT):
                        nc.tensor.matmul(mlp_ps[:, dc, :],
                                         lhsT=w2T[:, ff, dc * 128:(dc + 1) * 128],
                                         rhs=gT[:, ff, :],
                                         start=(ff == 0), stop=(ff == NFT - 1))
                mlp_sb = wk.tile([128, ND, S], F32, tag="arbuf", bufs=2, name="mlp_sb")
                for dc in range(ND):
                    nc.vector.tensor_copy(mlp_sb[:, dc, :], mlp_ps[:, dc, :])
                m_in = dram.tile([ND * 128 * S], F32, tag="m_in", bufs=2)
                m_out = dram.tile([ND * 128 * S], F32, tag="m_out", bufs=2, addr_space="Shared")
                nc.sync.dma_start(out=m_in[:].rearrange("(c p s) -> p c s", p=128, s=S),
                                  in_=mlp_sb[:])
                all_reduce(nc, [m_in[:]], [m_out[:]])
                mlp2 = wk.tile([128, ND, S], F32, tag="arbuf", bufs=2, name="mlp2")
                nc.sync.dma_start(out=mlp2[:],
                                  in_=m_out[:].rearrange("(c p s) -> p c s", p=128, s=S))
                for dc in range(ND):
                    nc.vector.tensor_tensor(out=fT[b][:, dc, :], in0=fT[b][:, dc, :],
                                            in1=mlp2[:, dc, :], op=ALU.add)

            # -------- layer 0 (f=0: R uniform, ex_wte = colmean(wte)) --------
            for b in range(B):
                exb_ps = psp.tile([128, S], F32, tag="tr", bufs=2, name="exb_ps")
                nc.tensor.matmul(exb_ps[:], lhsT=ones_row[:], rhs=ex0_row[:],
                                 start=True, stop=True)
                V_sb = wk.tile([128, NST, D], F32R, tag="V", bufs=1, name="V0")
                for st in range(NST):
                    nc.vector.tensor_tensor(out=V_sb[:, st, :], in0=e_sb[b][:, st, :],
                                            in1=exb_ps[:], op=ALU.subtract)
                sm_tail(b, 0, V_sb)

            # ---------------- layers 1..3 ----------------
            for k in range(1, NL):
                nd_out = {}
                for b in range(B):
                    num_ps = psp.tile([128, NST, D], F32, tag="acc4", bufs=1, name="num_ps")
                    den_ps = psp.tile([128, NST, 2], F32, tag="tr", bufs=2, name="den_ps")
                        def emit_B(Es_p, wt_p, vprev):
                        first, last = (vprev == 0), (vprev == NVT - 1)
                        for st in range(NST):
                            nc.tensor.matmul(num_ps[:, st, :],
                                             lhsT=Es_p[:, st * 128:(st + 1) * 128],
                                             rhs=wt_p[:], start=first, stop=last,
                                             skip_group_check=True)
                            nc.tensor.matmul(den_ps[:, st, :],
                                             lhsT=Es_p[:, st * 128:(st + 1) * 128],
                                             rhs=ones2[:], start=first, stop=last,
                                             skip_group_check=True)

                    pending = None
                    for v in range(NVT):
                        wt = wk.tile([128, D], F32R, tag="wt", bufs=3)
                        nc.sync.dma_start(out=wt[:], in_=d_wte[v * 128:(v + 1) * 128, :])
                        if pending is not None:
                            emit_B(*pending)
                        z_ps = psp.tile([128, S], F32, tag="zp", bufs=2, name="z_ps")
                        for dc in range(ND):
                            nc.tensor.matmul(z_ps[:], lhsT=wteT[:, dc, v * 128:(v + 1) * 128],
                                             rhs=fT[b][:, dc, :],
                                             start=(dc == 0), stop=(dc == ND - 1))
                        E = wk.tile([128, S], F32R, tag="E", bufs=2)
                        rs = wk.tile([128, 1], F32, tag="rs", bufs=3)
                        nc.scalar.activation(E[:], z_ps[:], AF.Exp, accum_out=rs[:])
                        rm = wk.tile([128, 1], F32, tag="rm", bufs=3)
                        nc.vector.reciprocal(rm[:], rs[:])
                        rm2 = wk.tile([128, 1], F32, tag="rm2", bufs=3)
                        nc.vector.tensor_tensor(out=rm2[:], in0=rm[:],
                                                in1=vmask[:, v:v + 1], op=ALU.mult)
                        Es = wk.tile([128, S], F32R, tag="Es", bufs=2)
                        nc.vector.tensor_scalar(out=Es[:], in0=E[:], scalar1=rm2[:, :1],
                                                scalar2=None, op0=ALU.mult)
                        pending = (Es, wt, v)
                    emit_B(*pending)
                    nsb = wk.tile([128, NST, D], F32, tag="arbuf", bufs=2, name="nsb")
                    for st in range(NST):
                        nc.vector.tensor_copy(nsb[:, st, :], num_ps[:, st, :])
                    dsb = wk.tile([128, NST], F32, tag="dsb", bufs=2)
                    nc.vector.tensor_copy(dsb[:], den_ps[:, :, 0])
                    nd_in = dram.tile([NST * 128 * D + S], F32, tag="nd_in", bufs=2)
                    ndo = dram.tile([NST * 128 * D + S], F32, tag="nd_out", bufs=2,
                                    addr_space="Shared")
                    nc.sync.dma_start(
                        out=nd_in[:NST * 128 * D].rearrange("(c p s) -> p c s", p=128, s=D),
                        in_=nsb[:])
                    nc.sync.dma_start(
                        out=nd_in[NST * 128 * D:].rearrange("(c p) -> p c", p=128),
                        in_=dsb[:])
                    all_reduce(nc, [nd_in[:]], [ndo[:]])
                    nd_out[b] = ndo
                for b in range(B):
                    ndo = nd_out[b]
                    n2 = wk.tile([128, NST, D], F32, tag="arbuf", bufs=2, name="n2")
                    nc.sync.dma_start(
                        out=n2[:],
                        in_=ndo[:NST * 128 * D].rearrange("(c p s) -> p c s", p=128, s=D))
                    d2 = wk.tile([128, NST], F32, tag="d2", bufs=2)
                    nc.sync.dma_start(
                        out=d2[:], in_=ndo[NST * 128 * D:].rearrange("(c p) -> p c", p=128))
                    rden = wk.tile([128, NST], F32, tag="rden", bufs=2)
                    nc.vector.reciprocal(rden[:], d2[:])
                    V_sb = wk.tile([128, NST, D], F32R, tag="V", bufs=1, name="V_sb")
                    for st in range(NST):
                        exw = wk.tile([128, D], F32, tag="exw", bufs=1)
                        nc.vector.tensor_scalar(out=exw[:], in0=n2[:, st, :],
                                                scalar1=rden[:, st:st + 1], scalar2=None,
                                                op0=ALU.mult)
                        nc.vector.tensor_tensor(out=V_sb[:, st, :], in0=e_sb[b][:, st, :],
                                                in1=exw[:], op=ALU.subtract)
                    sm_tail(b, k, V_sb)

            # ---------------- final LN + logits ----------------
            xl = wk.tile([128, ND, B], F32R, tag="xl")
            for b in range(B):
                xn = wk.tile([128, ND, S], F32R, tag="V", bufs=1, name="xn")
                ln_apply(b, lnf, xn)
                for dc in range(ND):
                    nc.vector.tensor_copy(xl[:, dc, b:b + 1], xn[:, dc, S - 1:S])
            for v8 in range(VS // 512):
                lg_ps = psp.tile([B, 512], F32, tag="zp", bufs=2, name="lg_ps")
                for dc in range(ND):
                    nc.tensor.matmul(lg_ps[:], lhsT=xl[:, dc, :],
                                     rhs=wteT[:, dc, v8 * 512:(v8 + 1) * 512],
                                     start=(dc == 0), stop=(dc == ND - 1))
                lg_sb = wk.tile([B, 512], F32, tag="lgsb", bufs=2)
                nc.vector.tensor_copy(lg_sb[:], lg_ps[:])
                nc.sync.dma_start(out=d_out[:, v8 * 512:(v8 + 1) * 512], in_=lg_sb[:])

    nc.compile()
    return nc


def _prep_inputs(x, wte, wpe, w_o, w1, w2, ln_mlp_w, ln_f_w):
    """Host-side sharding / layout prep (pure slicing, padding, transposes)."""
    x = np.asarray(x, dtype=np.int32)
    wte = np.ascontiguousarray(np.asarray(wte, dtype=np.float32))
    wpe = np.asarray(wpe, dtype=np.float32)
    w_o = np.asarray(w_o, dtype=np.float32)
    w1 = np.asarray(w1, dtype=np.float32)
    w2 = np.asarray(w2, dtype=np.float32)
    ln_mlp_w = np.asarray(ln_mlp_w, dtype=np.float32)
    ln_f_w = np.asarray(ln_f_w, dtype=np.float32)

    wte_pad = np.zeros((VP, D), dtype=np.float32)
    wte_pad[:V] = wte
    wteT_pad = np.ascontiguousarray(wte_pad.T)          # [D, VP]
    wpeT = np.ascontiguousarray(wpe[:S + 1].T)          # [D, S+1]
    w_oT = np.ascontiguousarray(w_o.T)                  # [8D, D]
    w1T = np.ascontiguousarray(w1.T)                    # [D, DFF]
    w2T = np.ascontiguousarray(w2.T)                    # [DFF, D]

    in_maps = []
    for c in range(NCORES):
        v0, v1 = c * VS, (c + 1) * VS
        f0, f1 = c * FFS, (c + 1) * FFS
        mask = np.zeros(VS, dtype=np.float32)
        nreal = min(max(V - v0, 0), VS)
        mask[:nreal] = 1.0
        in_maps.append({
            "wteT": np.ascontiguousarray(wteT_pad[:, v0:v1]),
            "wte_s": np.ascontiguousarray(wte_pad[v0:v1]),
            "wte_full": wte,
            "wpeT": wpeT,
            "w_oT": w_oT,
            "w1T_s": np.ascontiguousarray(w1T[:, f0:f1]),
            "w2T_s": np.ascontiguousarray(w2T[f0:f1]),
            "ln_mlp": ln_mlp_w,
            "ln_f": ln_f_w,
            "x_b0": np.ascontiguousarray(x[0]),
            "x_b1": np.ascontiguousarray(x[1]),
            "vmask": mask,
        })
    return in_maps


def kernel(x, wte, wpe, w_o, w1, w2, ln_mlp_w, ln_f_w, _trace=False):
    if "nc" not in _cached:
        _cached["nc"] = _build()
    nc = _cached["nc"]
    in_maps = _prep_inputs(x, wte, wpe, w_o, w1, w2, ln_mlp_w, ln_f_w)
    res = bass_utils.run_bass_kernel_spmd(nc, in_maps, core_ids=list(range(NCORES)),
                                          trace=_trace)
    _cached["last_result"] = res
    logits = np.concatenate([r["logits"] for r in res.results], axis=1)  # [B, VP]
    return logits[:, None, :V].astype(np.float32)
